# revision 7
# baseline (speedup 1.0000x reference)
"""Haar DWT2 (pywt-style) Trainium2 kernel.

Input : image_tensor [16, 1, 2048, 2048] f32
Output: (low [16, 1, 1024, 1024], high [16, 3, 1024, 1024]) f32
  LL=(a+b+c+d)/2, LH=(a+b-c-d)/2, HL=(a-b+c-d)/2, HH=(a-b-c+d)/2
  with a=x[2i,2j], b=x[2i,2j+1], c=x[2i+1,2j], d=x[2i+1,2j+1].

Sharding: pure data parallel — batch 16 split 2-per-core across 8 cores.

Per-core algorithm (per macro-tile of 256 input rows):
  - DMA [128, 4096] tile: partition p holds rows 2p and 2p+1 concatenated
    (16 KB contiguous per partition -> full-rate DMA).
  - ScalarE in-place *0.5 prescale.
  - VectorE row butterfly:  S|D tile, S = rows_even + rows_odd, D = sub.
  - VectorE col butterfly over stride-2 slices: OUTP = [LL|LH], OUTM = [HL|HH].
  - 4 contiguous 512 KB output DMAs.
"""

from contextlib import ExitStack

import numpy as np

import concourse.bass as bass
import concourse.mybir as mybir
import concourse.tile as tile
from concourse.bass_utils import run_bass_kernel_spmd

N_CORES = 8
B_FULL = 16
B_PER = B_FULL // N_CORES  # 2
H = W = 2048
HO = WO = 1024
P = 128
ROWS_PER_TILE = 2 * P          # 256 input rows per macro-tile
N_TILES = H // ROWS_PER_TILE   # 8 per batch
F32 = mybir.dt.float32


def _build_module() -> bass.Bass:
    nc = bass.Bass("TRN2", target_bir_lowering=False, debug=False,
                   num_devices=N_CORES)
    x = nc.dram_tensor("x", [B_PER, H, W], F32, kind="ExternalInput").ap()
    low = nc.dram_tensor("low", [B_PER, 1, HO, WO], F32,
                         kind="ExternalOutput").ap()
    high = nc.dram_tensor("high", [B_PER, 3, HO, WO], F32,
                          kind="ExternalOutput").ap()

    # [B, n, p, (t w)]: partition p of block n = rows n*256+2p, n*256+2p+1
    x_v = x.rearrange("b (n p t) w -> b n p (t w)", p=P, t=2)

    with tile.TileContext(nc) as tc, ExitStack() as ctx:
        in_pool = ctx.enter_context(tc.tile_pool(name="in", bufs=3))
        sd_pool = ctx.enter_context(tc.tile_pool(name="sd", bufs=2))
        out_pool = ctx.enter_context(tc.tile_pool(name="out", bufs=3))
        for b in range(B_PER):
            for n in range(N_TILES):
                t_in = in_pool.tile([P, 2 * W], F32)
                nc.gpsimd.dma_start(t_in[:], x_v[b, n])
                nc.scalar.mul(t_in[:], t_in[:], 0.5)

                sd = sd_pool.tile([P, 2 * W], F32)
                nc.vector.tensor_add(sd[:, 0:W], t_in[:, 0:W], t_in[:, W:])
                nc.vector.tensor_sub(sd[:, W:], t_in[:, 0:W], t_in[:, W:])

                outp = out_pool.tile([P, W], F32, tag="outp")
                outm = out_pool.tile([P, W], F32, tag="outm")
                nc.vector.tensor_add(outp[:], sd[:, 0::2], sd[:, 1::2])
                nc.vector.tensor_sub(outm[:], sd[:, 0::2], sd[:, 1::2])

                r0 = n * P
                nc.gpsimd.dma_start(low[b, 0, r0:r0 + P, :], outp[:, 0:WO])
                nc.gpsimd.dma_start(high[b, 0, r0:r0 + P, :], outp[:, WO:])
                nc.gpsimd.dma_start(high[b, 1, r0:r0 + P, :], outm[:, 0:WO])
                nc.gpsimd.dma_start(high[b, 2, r0:r0 + P, :], outm[:, WO:])
    return nc


def _hoist_extra_waits(nc: bass.Bass) -> None:
    """The bass2jax/walrus path encodes exactly ONE semaphore wait per ISA
    instruction (single EVENTS block); Tile's scheduler can attach 2+.
    Hoist the extras onto standalone sequencer wait instructions
    (EventSemaphore) on the same engine, immediately before the gated
    instruction — per-engine program order makes them happen-before it,
    so the dependency graph is unchanged (only ever more conservative)."""
    n = 0
    for fn in nc.m.functions:
        for blk in fn.blocks:
            insts = blk.instructions
            out = []
            for inst in insts:
                si = inst.sync_info
                if si is not None and si.on_wait and len(si.on_wait) > 1:
                    waits = list(si.on_wait)
                    for w in waits[:-1]:
                        ev = mybir.InstEventSemaphore(
                            name=f"{inst.name}-hw{n}",
                            engine=inst.engine,
                            ins=[],
                            outs=[],
                            sync_info=mybir.SyncInfo(on_wait=[w],
                                                     on_update=[]),
                        )
                        out.append(ev)
                        n += 1
                    inst.sync_info = mybir.SyncInfo(
                        on_wait=[waits[-1]], on_update=list(si.on_update))
                out.append(inst)
            if len(out) != len(insts):
                insts.clear()
                insts.extend(out)


_module_cache: bass.Bass | None = None


def _get_module() -> bass.Bass:
    global _module_cache
    if _module_cache is None:
        _module_cache = _build_module()
        _hoist_extra_waits(_module_cache)
    return _module_cache


def kernel(image_tensor: np.ndarray, _trace: bool = False):
    x = np.ascontiguousarray(image_tensor[:, 0]).astype(np.float32, copy=False)
    nc = _get_module()
    in_maps = [{"x": x[i * B_PER:(i + 1) * B_PER]} for i in range(N_CORES)]
    res = run_bass_kernel_spmd(nc, in_maps, list(range(N_CORES)),
                               trace=_trace)
    low = np.concatenate([r["low"] for r in res.results], axis=0)
    high = np.concatenate([r["high"] for r in res.results], axis=0)
    kernel.last_result = res
    return low, high
